# revision 11
# baseline (speedup 1.0000x reference)
"""Trainium2 Bass kernel for nn_DilationSpconv (3x sparse-conv + BN + ReLU).

Strategy: the voxel set is ~87.6% dense on a (batch, 353, 97) grid, so we
densify on the host and turn the sparse gather-conv into a dense 3x3 conv
implemented with shifted-slice matmuls (no per-element gathers on device).

Sharding: 8 cores = 4 scenes x 2 x-halves. Each core holds its half-scene
plus a 3-column x halo (recompute) -> fully independent cores, no
collectives.

Layout ("interleave-2"): layer tensor XI[128, W]: partition rows 0:64 hold
channels of even grid-sites, rows 64:128 hold channels of odd sites, column
j holds sites (2j, 2j+1). A 128x128 stationary weight block then packs 2x2
(input-parity x output-parity) 64x64 conv-offset blocks, and one matmul
computes 1024 sites' partial outputs with 128-deep contraction. 6 matmuls
cover all 9 offsets of a 3x3 kernel (75% PE utilization). Per-layer phase
shifts (phi = 3,2,1,0) keep the offset runs {g, g+1, g+2} even-aligned so
the 6-matmul covering works for every dx group.

v2 over the original baseline:
 - YP=98 (shared single pad row between adjacent x-columns) instead of 100
   -> 9016 grid columns instead of 9200 (+16 pad).
 - Per-layer shrinking column ranges (each layer only computes what the
   next layer reads; L3 only the owned output range) with a narrowed final
   window -> 26316 output columns/iter instead of 27648.
 - Next-iteration input DMAs (xi0, masks) are issued mid-body right after
   their last reader, so the loop back-edge exposes no DMA latency.
   Weights/BN vectors are loop-invariant and stay resident.
 - PE warmup runs once before the loop (cold-start only), not per
   iteration.
 - For_i back-edge uses branch-prefetch hints (PE body > 256 insts).

BN+ReLU fused into one ACT op (per-partition scale/bias); occupancy mask
(required so inactive/pad sites stay exactly zero between layers) is one
DVE multiply. Output stored fp16, widened to f32 on host.
"""

import os
import sys

import numpy as np

for _p in ("/opt/trn_rl_repo", "/opt/pypackages"):
    if os.path.isdir(_p) and _p not in sys.path:
        sys.path.append(_p)

# ---- problem constants (hardcoded, spec: nn_DilationSpconv_7370163880515) ----
N = 120000
C = 64
B = 4
XLIM = 352
YLIM = 96
EPS = 1e-5
NXS = 353   # x grid steps:  x in [-352, 352] step 2
NYS = 97    # y grid steps:  y in [-96, 96] step 2
YP = 98     # padded column height: row 0 pad, rows 1..97 real (pad row of
            # the next column doubles as this column's trailing pad)
NCORES = 8
OWN0 = 177          # x-cols owned by even cores (odd cores own 176)
NXL = 184           # local x columns in the per-core dense grid
GRIDC = NXL * YP // 2   # 9016 interleaved grid columns
MARG = 64           # lead margin (zero) in XI columns
WBUF = MARG + GRIDC + 96  # 9176 total XI columns
PHI = (3, 2, 1, 0)  # storage phase per layer tensor (delta-phi = +1 each layer)
# matmul column-shift offsets v, in order (dx=-1 j=0, dx=-1 j=1, dx=0 ...)
VOFF = (-49, -48, 0, 1, 49, 50)
WCOLS = 512         # matmul window width (PSUM bank = 512 fp32)
# per-layer output ranges in absolute XI columns [start, end): each layer
# computes only what its consumer reads (L3: owned outputs = grid cols
# [196, 8869) -> absolute [260, 8933); +-(49..50)+1 halo per layer up).
L1R = (162, 9033)
L2R = (211, 8983)
L3R = (260, 8933)
OUTW = L3R[1] - L3R[0]  # 8673 output columns DMA'd out (fp16)
OUT0 = L3R[0] - MARG    # first output grid column (196)

_CACHE = {}


def _windows(rng):
    b, e = rng
    out = []
    while b < e:
        w = min(WCOLS, e - b)
        out.append((b, w))
        b += w
    return out


def _core_geometry(core):
    half = core % 2
    x0 = 0 if half == 0 else OWN0
    own = OWN0 if half == 0 else NXS - OWN0
    xstart = x0 - 4  # local col L maps to global x-step xstart + L
    lo = max(0, x0 - 3)
    hi = min(NXS, x0 + own + 3)
    return x0, own, xstart, lo, hi


def _host_prepare(feat, coor, Ws, scales, biases, np_dt):
    """Build per-core dense interleaved grids, masks, weight stacks, BN vecs."""
    xs = (coor[:, 1].astype(np.int64) + XLIM) // 2  # [0, 353)
    ys = (coor[:, 2].astype(np.int64) + YLIM) // 2  # [0, 97)
    b = coor[:, 0].astype(np.int64)

    xi0 = np.zeros((NCORES, 128, WBUF), np.float32)
    m1 = np.zeros((NCORES, 128, WBUF), np.float32)
    m2 = np.zeros((NCORES, 128, WBUF), np.float32)
    ch = np.arange(C)

    for core in range(NCORES):
        scene = core // 2
        _, _, xstart, lo, hi = _core_geometry(core)
        sel = (b == scene) & (xs >= lo) & (xs < hi)
        L = xs[sel] - xstart
        s = L * YP + ys[sel] + 1
        # layer-0 features at phase 3
        q = s + PHI[0]
        rows = (q & 1) * 64
        cols = MARG + (q >> 1)
        xi0[core, rows[:, None] + ch[None, :], cols[:, None]] = feat[sel]
        # occupancy masks at phases 2 (layer-1 out) and 1 (layer-2 out)
        for mk, phi in ((m1, PHI[1]), (m2, PHI[2])):
            qq = s + phi
            mk[core, ((qq & 1) * 64)[:, None] + ch[None, :],
               (MARG + (qq >> 1))[:, None]] = 1.0

    # weight stacks: per layer, 6 stationaries of [contract 128, out 128]
    def k_of(dxs, dys):
        return 3 * (dxs + 1) + (dys + 1)

    mats = []
    for W in Ws:  # [9, 64, 64] (k, c_in, c_out)
        for dxs in (-1, 0, 1):
            for j in (0, 1):
                M = np.zeros((128, 128), np.float32)
                if j == 0:
                    M[0:64, 0:64] = W[k_of(dxs, -1)]      # A: even-in -> even-out
                    M[64:128, 0:64] = W[k_of(dxs, 0)]     # C: odd-in  -> even-out
                    M[64:128, 64:128] = W[k_of(dxs, -1)]  # D: odd-in  -> odd-out
                else:
                    M[0:64, 0:64] = W[k_of(dxs, 1)]       # A
                    M[0:64, 64:128] = W[k_of(dxs, 0)]     # B: even-in -> odd-out
                    M[64:128, 64:128] = W[k_of(dxs, 1)]   # D
                mats.append(M)
    wstack = np.stack(mats).transpose(1, 0, 2).reshape(128, 18 * 128)

    bnv = np.zeros((128, 8), np.float32)
    for l in range(3):
        bnv[0:64, l] = scales[l]
        bnv[64:128, l] = scales[l]
        bnv[0:64, 3 + l] = biases[l]
        bnv[64:128, 3 + l] = biases[l]

    mask_dt = np.float16 if np_dt == np.float16 else _BF16
    return (xi0.astype(np_dt), m1.astype(mask_dt),
            m2.astype(mask_dt), wstack.astype(np_dt), bnv)


def _build_program(dt_key, loop_n=0, variant="v2", psum_bufs=8, warmup=10,
                   hints=False, stagger=False, unroll=8, stage_marks=True):
    import concourse.tile as tile
    from concourse import bacc, mybir

    f32 = mybir.dt.float32
    f16 = mybir.dt.float16
    if dt_key == "bf16":
        DT = mybir.dt.bfloat16
        BF = mybir.dt.bfloat16
    elif dt_key == "fp16":
        DT = mybir.dt.float16
        BF = mybir.dt.float16
    else:
        raise ValueError(dt_key)

    nc = bacc.Bacc("TRN2", target_bir_lowering=False, debug=False,
                   num_devices=NCORES)
    xi0_d = nc.dram_tensor("xi0", [128, WBUF], DT, kind="ExternalInput").ap()
    m1_d = nc.dram_tensor("m1", [128, WBUF], BF, kind="ExternalInput").ap()
    m2_d = nc.dram_tensor("m2", [128, WBUF], BF, kind="ExternalInput").ap()
    wts_d = nc.dram_tensor("wts", [128, 18 * 128], DT, kind="ExternalInput").ap()
    bnv_d = nc.dram_tensor("bnv", [128, 8], f32, kind="ExternalInput").ap()
    out_d = nc.dram_tensor("out", [128, OUTW], f16, kind="ExternalOutput").ap()

    Relu = mybir.ActivationFunctionType.Relu
    mult = mybir.AluOpType.mult

    WIN = [_windows(L1R), _windows(L2R), _windows(L3R)]

    with tile.TileContext(nc) as tc:
        with (
            tc.tile_pool(name="big", bufs=1) as big,
            tc.tile_pool(name="psum", bufs=psum_bufs, space="PSUM") as psump,
            tc.tile_pool(name="tmp", bufs=6) as tmpp,
        ):
            xa = big.tile([128, WBUF], DT)
            xb = big.tile([128, WBUF], DT)
            xc = big.tile([128, WBUF], DT)
            x3 = big.tile([128, OUTW], f16)
            m1t = big.tile([128, WBUF], BF)
            m2t = big.tile([128, WBUF], BF)
            wt = big.tile([128, 18 * 128], DT)
            bnt = big.tile([128, 8], f32)
            scr = big.tile([128, 640], DT)

            def dma_chunks(dst, src, edges, eng):
                for a, bnd in zip(edges[:-1], edges[1:]):
                    eng.dma_start(out=dst[:, a:bnd], in_=src[:, a:bnd])

            # xi0 chunk edges: leading chunks small so L1 window 0 (reads
            # cols [113, 724)) can start early.
            xi_edges = [0, 760, 1480, 2560, 3680, 4800, 5920, 7040, 8160, WBUF]
            mk_edges = [0, 2294, 4588, 6882, WBUF]

            def prologue():
                nc.sync.dma_start(out=bnt, in_=bnv_d)
                nc.sync.dma_start(out=wt, in_=wts_d)
                dma_chunks(xa, xi0_d, xi_edges, nc.sync)
                dma_chunks(m1t, m1_d, mk_edges, nc.sync)
                dma_chunks(m2t, m2_d, mk_edges, nc.sync)
                nc.vector.memset(scr, 0.0)
                for _ in range(warmup):
                    wps = psump.tile([128, WCOLS], f32, tag="ps")
                    nc.tensor.matmul(wps, scr[:, 0:128], scr[:, 128:640],
                                     start=True, stop=True)

            def layer(xin, xout, mt, l, prefetch=None, boundary_after=None):
                sc = bnt[:, l:l + 1]
                bi = bnt[:, 3 + l:4 + l]
                wins = WIN[l]
                for wi, (base, wc) in enumerate(wins):
                    if wi == boundary_after:
                        tc.stage_boundary()
                    ps = psump.tile([128, WCOLS], f32, tag="ps")
                    for i, v in enumerate(VOFF):
                        lhsT = wt[:, (6 * l + i) * 128:(6 * l + i + 1) * 128]
                        nc.tensor.matmul(ps[:, 0:wc], lhsT,
                                         xin[:, base + v:base + v + wc],
                                         start=(i == 0), stop=(i == 5))
                    if mt is not None:
                        tm = tmpp.tile([128, WCOLS], DT, tag="tm")
                        nc.scalar.activation(tm[:, 0:wc], ps[:, 0:wc], Relu,
                                             bias=bi, scale=sc)
                        nc.vector.tensor_tensor(
                            out=xout[:, base:base + wc], in0=tm[:, 0:wc],
                            in1=mt[:, base:base + wc], op=mult)
                    else:
                        o0 = base - L3R[0]
                        nc.scalar.activation(x3[:, o0:o0 + wc], ps[:, 0:wc],
                                             Relu, bias=bi, scale=sc)
                        nc.sync.dma_start(out=out_d[:, o0:o0 + wc],
                                          in_=x3[:, o0:o0 + wc])
                if prefetch is not None:
                    prefetch()

            def body(prefetch, staged=False):
                def pf1():
                    if prefetch:
                        dma_chunks(m1t, m1_d, mk_edges, nc.sync)
                        dma_chunks(xa, xi0_d, xi_edges, nc.sync)
                    if staged:
                        tc.stage_boundary()

                def pf2():
                    if prefetch:
                        dma_chunks(m2t, m2_d, mk_edges, nc.sync)
                    if staged:
                        tc.stage_boundary()

                layer(xa, xb, m1t, 0, pf1,
                      boundary_after=9 if staged else None)
                layer(xb, xc, m2t, 1, pf2)
                layer(xc, None, None, 2)

            prologue()
            if loop_n > 0:
                if loop_n % unroll != 0:
                    unroll = 1
                he = (mybir.EngineType.PE,) if hints else ()
                with tc.For_i(0, loop_n // unroll, 1, hint_engines=he,
                              staggered_reset=stagger):
                    for _ in range(unroll):
                        body(prefetch=True, staged=stagger and stage_marks)
            else:
                body(prefetch=False)
    nc.compile()
    return nc


def _get_np_dt(dt_key):
    if dt_key == "bf16":
        import ml_dtypes
        return ml_dtypes.bfloat16
    if dt_key == "fp16":
        return np.float16
    return np.float32


def kernel(feat, coor, kin_idx,
           W1, g1, b1, m1, v1,
           W2, g2, b2, m2, v2,
           W3, g3, b3, m3, v3):
    from concourse import bass_utils

    dt_key = os.environ.get("KERNEL_DT", "fp16")
    np_dt = _get_np_dt(dt_key)

    feat = np.asarray(feat, np.float32)
    coor = np.asarray(coor)
    Ws = [np.asarray(W, np.float32) for W in (W1, W2, W3)]
    scales, biases = [], []
    for g, bb, mm, vv in ((g1, b1, m1, v1), (g2, b2, m2, v2), (g3, b3, m3, v3)):
        s = np.asarray(g, np.float32) / np.sqrt(np.asarray(vv, np.float32) + EPS)
        scales.append(s)
        biases.append(np.asarray(bb, np.float32) - np.asarray(mm, np.float32) * s)

    xi0, m1g, m2g, wstack, bnv = _host_prepare(feat, coor, Ws, scales, biases,
                                               np_dt)

    if dt_key not in _CACHE:
        _CACHE[dt_key] = _build_program(dt_key)
    nc = _CACHE[dt_key]

    in_maps = [
        {"xi0": np.ascontiguousarray(xi0[c]),
         "m1": np.ascontiguousarray(m1g[c]),
         "m2": np.ascontiguousarray(m2g[c]),
         "wts": wstack, "bnv": bnv}
        for c in range(NCORES)
    ]
    res = None
    for attempt in range(3):
        try:
            res = bass_utils.run_bass_kernel_spmd(
                nc, in_maps, core_ids=list(range(NCORES)))
            break
        except Exception:
            if attempt == 2:
                raise
            import time
            time.sleep(5)
    grids = np.stack([np.asarray(r["out"], np.float32)
                      for r in res.results])  # [8, 128, OUTW]
    return _gather(grids, coor)


def _gather(grids, coor):
    """Gather per-voxel rows from the owning core's grid (phase 0)."""
    grids = grids.reshape(NCORES, 2, 64, OUTW)
    xs = (coor[:, 1].astype(np.int64) + XLIM) // 2
    ys = (coor[:, 2].astype(np.int64) + YLIM) // 2
    b = coor[:, 0].astype(np.int64)
    half = (xs >= OWN0).astype(np.int64)
    core = 2 * b + half
    xstart = np.where(half == 0, -4, OWN0 - 4)
    s = (xs - xstart) * YP + ys + 1
    out = grids[core, s & 1, :, (s >> 1) - OUT0].astype(np.float32)  # [N, 64]

    xy_ok = ((coor[:, 1] > -XLIM) & (coor[:, 1] <= XLIM)
             & (coor[:, 2] > -YLIM) & (coor[:, 2] <= YLIM))
    out *= xy_ok[:, None].astype(np.float32)
    return out


_BF16 = None


def _init_bf16():
    global _BF16
    import ml_dtypes
    _BF16 = ml_dtypes.bfloat16


_init_bf16()


# revision 12
# speedup vs baseline: 1.0537x; 1.0537x over previous
"""Trainium2 Bass kernel for nn_DilationSpconv (3x sparse-conv + BN + ReLU).

Strategy: the voxel set is ~87.6% dense on a (batch, 353, 97) grid, so we
densify on the host and turn the sparse gather-conv into a dense 3x3 conv
implemented with shifted-slice matmuls (no per-element gathers on device).

Sharding: 8 cores = 4 scenes x 2 x-halves. Each core holds its half-scene
plus a 3-column x halo (recompute) -> fully independent cores, no
collectives.

Layout ("interleave-2"): layer tensor XI[128, W]: partition rows 0:64 hold
channels of even grid-sites, rows 64:128 hold channels of odd sites, column
j holds sites (2j, 2j+1). A 128x128 stationary weight block then packs 2x2
(input-parity x output-parity) 64x64 conv-offset blocks, and one matmul
computes 1024 sites' partial outputs with 128-deep contraction. 6 matmuls
cover all 9 offsets of a 3x3 kernel (75% PE utilization). Per-layer phase
shifts (phi = 3,2,1,0) keep the offset runs {g, g+1, g+2} even-aligned so
the 6-matmul covering works for every dx group.

v2 over the original baseline:
 - YP=98 (shared single pad row between adjacent x-columns) instead of 100
   -> 9016 grid columns instead of 9200 (+16 pad).
 - Per-layer shrinking column ranges (each layer only computes what the
   next layer reads; L3 only the owned output range) with a narrowed final
   window -> 26316 output columns/iter instead of 27648.
 - Next-iteration input DMAs (xi0, masks) are issued mid-body right after
   their last reader, so the loop back-edge exposes no DMA latency.
   Weights/BN vectors are loop-invariant and stay resident.
 - PE warmup runs once before the loop (cold-start only), not per
   iteration.
 - For_i back-edge uses branch-prefetch hints (PE body > 256 insts).

BN+ReLU fused into one ACT op (per-partition scale/bias); occupancy mask
(required so inactive/pad sites stay exactly zero between layers) is one
DVE multiply. Output stored fp16, widened to f32 on host.
"""

import os
import sys

import numpy as np

for _p in ("/opt/trn_rl_repo", "/opt/pypackages"):
    if os.path.isdir(_p) and _p not in sys.path:
        sys.path.append(_p)

# ---- problem constants (hardcoded, spec: nn_DilationSpconv_7370163880515) ----
N = 120000
C = 64
B = 4
XLIM = 352
YLIM = 96
EPS = 1e-5
NXS = 353   # x grid steps:  x in [-352, 352] step 2
NYS = 97    # y grid steps:  y in [-96, 96] step 2
YP = 98     # padded column height: row 0 pad, rows 1..97 real (pad row of
            # the next column doubles as this column's trailing pad)
NCORES = 8
OWN0 = 177          # x-cols owned by even cores (odd cores own 176)
NXL = 184           # local x columns in the per-core dense grid
GRIDC = NXL * YP // 2   # 9016 interleaved grid columns
MARG = 64           # lead margin (zero) in XI columns
WBUF = MARG + GRIDC + 96  # 9176 total XI columns
PHI = (3, 2, 1, 0)  # storage phase per layer tensor (delta-phi = +1 each layer)
# matmul column-shift offsets v, in order (dx=-1 j=0, dx=-1 j=1, dx=0 ...)
VOFF = (-49, -48, 0, 1, 49, 50)
WCOLS = 512         # matmul window width (PSUM bank = 512 fp32)
# per-layer output ranges in absolute XI columns [start, end): each layer
# computes only what its consumer reads (L3: owned outputs = grid cols
# [196, 8869) -> absolute [260, 8933); +-(49..50)+1 halo per layer up).
L1R = (162, 9033)
L2R = (211, 8983)
L3R = (260, 8933)
OUTW = L3R[1] - L3R[0]  # 8673 output columns DMA'd out (fp16)
OUT0 = L3R[0] - MARG    # first output grid column (196)

_CACHE = {}


def _windows(rng):
    b, e = rng
    out = []
    while b < e:
        w = min(WCOLS, e - b)
        out.append((b, w))
        b += w
    return out


def _core_geometry(core):
    half = core % 2
    x0 = 0 if half == 0 else OWN0
    own = OWN0 if half == 0 else NXS - OWN0
    xstart = x0 - 4  # local col L maps to global x-step xstart + L
    lo = max(0, x0 - 3)
    hi = min(NXS, x0 + own + 3)
    return x0, own, xstart, lo, hi


def _host_prepare(feat, coor, Ws, scales, biases, np_dt):
    """Build per-core dense interleaved grids, masks, weight stacks, BN vecs."""
    xs = (coor[:, 1].astype(np.int64) + XLIM) // 2  # [0, 353)
    ys = (coor[:, 2].astype(np.int64) + YLIM) // 2  # [0, 97)
    b = coor[:, 0].astype(np.int64)

    xi0 = np.zeros((NCORES, 128, WBUF), np.float32)
    m1 = np.zeros((NCORES, 128, WBUF), np.float32)
    m2 = np.zeros((NCORES, 128, WBUF), np.float32)
    ch = np.arange(C)

    for core in range(NCORES):
        scene = core // 2
        _, _, xstart, lo, hi = _core_geometry(core)
        sel = (b == scene) & (xs >= lo) & (xs < hi)
        L = xs[sel] - xstart
        s = L * YP + ys[sel] + 1
        # layer-0 features at phase 3
        q = s + PHI[0]
        rows = (q & 1) * 64
        cols = MARG + (q >> 1)
        xi0[core, rows[:, None] + ch[None, :], cols[:, None]] = feat[sel]
        # occupancy masks at phases 2 (layer-1 out) and 1 (layer-2 out)
        for mk, phi in ((m1, PHI[1]), (m2, PHI[2])):
            qq = s + phi
            mk[core, ((qq & 1) * 64)[:, None] + ch[None, :],
               (MARG + (qq >> 1))[:, None]] = 1.0

    # weight stacks: per layer, 6 stationaries of [contract 128, out 128]
    def k_of(dxs, dys):
        return 3 * (dxs + 1) + (dys + 1)

    mats = []
    for W in Ws:  # [9, 64, 64] (k, c_in, c_out)
        for dxs in (-1, 0, 1):
            for j in (0, 1):
                M = np.zeros((128, 128), np.float32)
                if j == 0:
                    M[0:64, 0:64] = W[k_of(dxs, -1)]      # A: even-in -> even-out
                    M[64:128, 0:64] = W[k_of(dxs, 0)]     # C: odd-in  -> even-out
                    M[64:128, 64:128] = W[k_of(dxs, -1)]  # D: odd-in  -> odd-out
                else:
                    M[0:64, 0:64] = W[k_of(dxs, 1)]       # A
                    M[0:64, 64:128] = W[k_of(dxs, 0)]     # B: even-in -> odd-out
                    M[64:128, 64:128] = W[k_of(dxs, 1)]   # D
                mats.append(M)
    wstack = np.stack(mats).transpose(1, 0, 2).reshape(128, 18 * 128)

    bnv = np.zeros((128, 8), np.float32)
    for l in range(3):
        bnv[0:64, l] = scales[l]
        bnv[64:128, l] = scales[l]
        bnv[0:64, 3 + l] = biases[l]
        bnv[64:128, 3 + l] = biases[l]

    mask_dt = np.float16 if np_dt == np.float16 else _BF16
    return (xi0.astype(np_dt), m1.astype(mask_dt),
            m2.astype(mask_dt), wstack.astype(np_dt), bnv)


def _build_program(dt_key, loop_n=0, variant="v2", psum_bufs=8, warmup=10,
                   hints=True, stagger=False, unroll=8, stage_marks=True):
    import concourse.tile as tile
    from concourse import bacc, mybir

    f32 = mybir.dt.float32
    f16 = mybir.dt.float16
    if dt_key == "bf16":
        DT = mybir.dt.bfloat16
        BF = mybir.dt.bfloat16
    elif dt_key == "fp16":
        DT = mybir.dt.float16
        BF = mybir.dt.float16
    else:
        raise ValueError(dt_key)

    nc = bacc.Bacc("TRN2", target_bir_lowering=False, debug=False,
                   num_devices=NCORES)
    xi0_d = nc.dram_tensor("xi0", [128, WBUF], DT, kind="ExternalInput").ap()
    m1_d = nc.dram_tensor("m1", [128, WBUF], BF, kind="ExternalInput").ap()
    m2_d = nc.dram_tensor("m2", [128, WBUF], BF, kind="ExternalInput").ap()
    wts_d = nc.dram_tensor("wts", [128, 18 * 128], DT, kind="ExternalInput").ap()
    bnv_d = nc.dram_tensor("bnv", [128, 8], f32, kind="ExternalInput").ap()
    out_d = nc.dram_tensor("out", [128, OUTW], f16, kind="ExternalOutput").ap()

    Relu = mybir.ActivationFunctionType.Relu
    mult = mybir.AluOpType.mult

    WIN = [_windows(L1R), _windows(L2R), _windows(L3R)]

    with tile.TileContext(nc) as tc:
        with (
            tc.tile_pool(name="big", bufs=1) as big,
            tc.tile_pool(name="psum", bufs=psum_bufs, space="PSUM") as psump,
            tc.tile_pool(name="tmp", bufs=6) as tmpp,
        ):
            xa = big.tile([128, WBUF], DT)
            xb = big.tile([128, WBUF], DT)
            xc = big.tile([128, WBUF], DT)
            x3 = big.tile([128, OUTW], f16)
            m1t = big.tile([128, WBUF], BF)
            m2t = big.tile([128, WBUF], BF)
            wt = big.tile([128, 18 * 128], DT)
            bnt = big.tile([128, 8], f32)
            scr = big.tile([128, 640], DT)

            def dma_chunks(dst, src, edges, eng):
                for a, bnd in zip(edges[:-1], edges[1:]):
                    eng.dma_start(out=dst[:, a:bnd], in_=src[:, a:bnd])

            # xi0 chunk edges: leading chunks small so L1 window 0 (reads
            # cols [113, 724)) can start early.
            xi_edges = [0, 760, 1480, 2560, 3680, 4800, 5920, 7040, 8160, WBUF]
            mk_edges = [0, 2294, 4588, 6882, WBUF]

            def prologue():
                nc.sync.dma_start(out=bnt, in_=bnv_d)
                nc.sync.dma_start(out=wt, in_=wts_d)
                dma_chunks(xa, xi0_d, xi_edges, nc.sync)
                dma_chunks(m1t, m1_d, mk_edges, nc.sync)
                dma_chunks(m2t, m2_d, mk_edges, nc.sync)
                nc.vector.memset(scr, 0.0)
                for _ in range(warmup):
                    wps = psump.tile([128, WCOLS], f32, tag="ps")
                    nc.tensor.matmul(wps, scr[:, 0:128], scr[:, 128:640],
                                     start=True, stop=True)

            def layer(xin, xout, mt, l, prefetch=None, boundary_after=None):
                sc = bnt[:, l:l + 1]
                bi = bnt[:, 3 + l:4 + l]
                wins = WIN[l]
                for wi, (base, wc) in enumerate(wins):
                    if wi == boundary_after:
                        tc.stage_boundary()
                    ps = psump.tile([128, WCOLS], f32, tag="ps")
                    for i, v in enumerate(VOFF):
                        lhsT = wt[:, (6 * l + i) * 128:(6 * l + i + 1) * 128]
                        nc.tensor.matmul(ps[:, 0:wc], lhsT,
                                         xin[:, base + v:base + v + wc],
                                         start=(i == 0), stop=(i == 5))
                    if mt is not None:
                        tm = tmpp.tile([128, WCOLS], DT, tag="tm")
                        nc.scalar.activation(tm[:, 0:wc], ps[:, 0:wc], Relu,
                                             bias=bi, scale=sc)
                        nc.vector.tensor_tensor(
                            out=xout[:, base:base + wc], in0=tm[:, 0:wc],
                            in1=mt[:, base:base + wc], op=mult)
                    else:
                        o0 = base - L3R[0]
                        nc.scalar.activation(x3[:, o0:o0 + wc], ps[:, 0:wc],
                                             Relu, bias=bi, scale=sc)
                        nc.sync.dma_start(out=out_d[:, o0:o0 + wc],
                                          in_=x3[:, o0:o0 + wc])
                if prefetch is not None:
                    prefetch()

            def body(prefetch, staged=False):
                def pf1():
                    if prefetch:
                        dma_chunks(m1t, m1_d, mk_edges, nc.sync)
                        dma_chunks(xa, xi0_d, xi_edges, nc.sync)
                    if staged:
                        tc.stage_boundary()

                def pf2():
                    if prefetch:
                        dma_chunks(m2t, m2_d, mk_edges, nc.sync)
                    if staged:
                        tc.stage_boundary()

                layer(xa, xb, m1t, 0, pf1,
                      boundary_after=9 if staged else None)
                layer(xb, xc, m2t, 1, pf2)
                layer(xc, None, None, 2)

            prologue()
            if loop_n > 0:
                if loop_n % unroll != 0:
                    unroll = 1
                he = (mybir.EngineType.PE,) if hints else ()
                with tc.For_i(0, loop_n // unroll, 1, hint_engines=he,
                              staggered_reset=stagger):
                    for _ in range(unroll):
                        body(prefetch=True, staged=stagger and stage_marks)
            else:
                body(prefetch=False)
    nc.compile()
    return nc


def _get_np_dt(dt_key):
    if dt_key == "bf16":
        import ml_dtypes
        return ml_dtypes.bfloat16
    if dt_key == "fp16":
        return np.float16
    return np.float32


def kernel(feat, coor, kin_idx,
           W1, g1, b1, m1, v1,
           W2, g2, b2, m2, v2,
           W3, g3, b3, m3, v3):
    from concourse import bass_utils

    dt_key = os.environ.get("KERNEL_DT", "fp16")
    np_dt = _get_np_dt(dt_key)

    feat = np.asarray(feat, np.float32)
    coor = np.asarray(coor)
    Ws = [np.asarray(W, np.float32) for W in (W1, W2, W3)]
    scales, biases = [], []
    for g, bb, mm, vv in ((g1, b1, m1, v1), (g2, b2, m2, v2), (g3, b3, m3, v3)):
        s = np.asarray(g, np.float32) / np.sqrt(np.asarray(vv, np.float32) + EPS)
        scales.append(s)
        biases.append(np.asarray(bb, np.float32) - np.asarray(mm, np.float32) * s)

    xi0, m1g, m2g, wstack, bnv = _host_prepare(feat, coor, Ws, scales, biases,
                                               np_dt)

    if dt_key not in _CACHE:
        _CACHE[dt_key] = _build_program(dt_key)
    nc = _CACHE[dt_key]

    in_maps = [
        {"xi0": np.ascontiguousarray(xi0[c]),
         "m1": np.ascontiguousarray(m1g[c]),
         "m2": np.ascontiguousarray(m2g[c]),
         "wts": wstack, "bnv": bnv}
        for c in range(NCORES)
    ]
    res = None
    for attempt in range(3):
        try:
            res = bass_utils.run_bass_kernel_spmd(
                nc, in_maps, core_ids=list(range(NCORES)))
            break
        except Exception:
            if attempt == 2:
                raise
            import time
            time.sleep(5)
    grids = np.stack([np.asarray(r["out"], np.float32)
                      for r in res.results])  # [8, 128, OUTW]
    return _gather(grids, coor)


def _gather(grids, coor):
    """Gather per-voxel rows from the owning core's grid (phase 0)."""
    grids = grids.reshape(NCORES, 2, 64, OUTW)
    xs = (coor[:, 1].astype(np.int64) + XLIM) // 2
    ys = (coor[:, 2].astype(np.int64) + YLIM) // 2
    b = coor[:, 0].astype(np.int64)
    half = (xs >= OWN0).astype(np.int64)
    core = 2 * b + half
    xstart = np.where(half == 0, -4, OWN0 - 4)
    s = (xs - xstart) * YP + ys + 1
    out = grids[core, s & 1, :, (s >> 1) - OUT0].astype(np.float32)  # [N, 64]

    xy_ok = ((coor[:, 1] > -XLIM) & (coor[:, 1] <= XLIM)
             & (coor[:, 2] > -YLIM) & (coor[:, 2] <= YLIM))
    out *= xy_ok[:, None].astype(np.float32)
    return out


_BF16 = None


def _init_bf16():
    global _BF16
    import ml_dtypes
    _BF16 = ml_dtypes.bfloat16


_init_bf16()


# revision 13
# speedup vs baseline: 1.0669x; 1.0125x over previous
"""Trainium2 Bass kernel for nn_DilationSpconv (3x sparse-conv + BN + ReLU).

Strategy: the voxel set is ~87.6% dense on a (batch, 353, 97) grid, so we
densify on the host and turn the sparse gather-conv into a dense 3x3 conv
implemented with shifted-slice matmuls (no per-element gathers on device).

Sharding: 8 cores = 4 scenes x 2 x-halves. Each core holds its half-scene
plus a 3-column x halo (recompute) -> fully independent cores, no
collectives.

Layout ("interleave-2"): layer tensor XI[128, W]: partition rows 0:64 hold
channels of even grid-sites, rows 64:128 hold channels of odd sites, column
j holds sites (2j, 2j+1). A 128x128 stationary weight block then packs 2x2
(input-parity x output-parity) 64x64 conv-offset blocks, and one matmul
computes 1024 sites' partial outputs with 128-deep contraction. 6 matmuls
cover all 9 offsets of a 3x3 kernel (75% PE utilization). Per-layer phase
shifts (phi = 3,2,1,0) keep the offset runs {g, g+1, g+2} even-aligned so
the 6-matmul covering works for every dx group.

v2 over the original baseline:
 - YP=98 (shared single pad row between adjacent x-columns) instead of 100
   -> 9016 grid columns instead of 9200 (+16 pad).
 - Per-layer shrinking column ranges (each layer only computes what the
   next layer reads; L3 only the owned output range) with a narrowed final
   window -> 26316 output columns/iter instead of 27648.
 - Next-iteration input DMAs (xi0, masks) are issued mid-body right after
   their last reader, so the loop back-edge exposes no DMA latency.
   Weights/BN vectors are loop-invariant and stay resident.
 - PE warmup runs once before the loop (cold-start only), not per
   iteration.
 - Timing-loop body is unrolled 8x inside For_i (amortizes the ~2us
   all-engine back-edge barrier + tail drain) with PE branch-prefetch
   hints (8x body is far past the 256-inst IRAM block).

BN+ReLU fused into one ACT op (per-partition scale/bias); occupancy mask
(required so inactive/pad sites stay exactly zero between layers) is one
DVE multiply. Output stored fp16, widened to f32 on host.
"""

import os
import sys

import numpy as np

for _p in ("/opt/trn_rl_repo", "/opt/pypackages"):
    if os.path.isdir(_p) and _p not in sys.path:
        sys.path.append(_p)

# ---- problem constants (hardcoded, spec: nn_DilationSpconv_7370163880515) ----
N = 120000
C = 64
B = 4
XLIM = 352
YLIM = 96
EPS = 1e-5
NXS = 353   # x grid steps:  x in [-352, 352] step 2
NYS = 97    # y grid steps:  y in [-96, 96] step 2
YP = 98     # padded column height: row 0 pad, rows 1..97 real (pad row of
            # the next column doubles as this column's trailing pad)
NCORES = 8
OWN0 = 177          # x-cols owned by even cores (odd cores own 176)
NXL = 184           # local x columns in the per-core dense grid
GRIDC = NXL * YP // 2   # 9016 interleaved grid columns
MARG = 64           # lead margin (zero) in XI columns
WBUF = MARG + GRIDC + 96  # 9176 total XI columns
PHI = (3, 2, 1, 0)  # storage phase per layer tensor (delta-phi = +1 each layer)
# matmul column-shift offsets v, in order (dx=-1 j=0, dx=-1 j=1, dx=0 ...)
VOFF = (-49, -48, 0, 1, 49, 50)
WCOLS = 512         # matmul window width (PSUM bank = 512 fp32)
# per-layer output ranges in absolute XI columns [start, end): each layer
# computes only what its consumer reads (L3: owned outputs = grid cols
# [196, 8869) -> absolute [260, 8933); +-(49..50)+1 halo per layer up).
L1R = (162, 9033)
L2R = (211, 8983)
L3R = (260, 8933)
OUTW = L3R[1] - L3R[0]  # 8673 output columns DMA'd out (fp16)
OUT0 = L3R[0] - MARG    # first output grid column (196)

_CACHE = {}


def _windows(rng):
    b, e = rng
    out = []
    while b < e:
        w = min(WCOLS, e - b)
        out.append((b, w))
        b += w
    return out


def _core_geometry(core):
    half = core % 2
    x0 = 0 if half == 0 else OWN0
    own = OWN0 if half == 0 else NXS - OWN0
    xstart = x0 - 4  # local col L maps to global x-step xstart + L
    lo = max(0, x0 - 3)
    hi = min(NXS, x0 + own + 3)
    return x0, own, xstart, lo, hi


def _host_prepare(feat, coor, Ws, scales, biases, np_dt):
    """Build per-core dense interleaved grids, masks, weight stacks, BN vecs."""
    xs = (coor[:, 1].astype(np.int64) + XLIM) // 2  # [0, 353)
    ys = (coor[:, 2].astype(np.int64) + YLIM) // 2  # [0, 97)
    b = coor[:, 0].astype(np.int64)

    xi0 = np.zeros((NCORES, 128, WBUF), np.float32)
    m1 = np.zeros((NCORES, 128, WBUF), np.float32)
    m2 = np.zeros((NCORES, 128, WBUF), np.float32)
    ch = np.arange(C)

    for core in range(NCORES):
        scene = core // 2
        _, _, xstart, lo, hi = _core_geometry(core)
        sel = (b == scene) & (xs >= lo) & (xs < hi)
        L = xs[sel] - xstart
        s = L * YP + ys[sel] + 1
        # layer-0 features at phase 3
        q = s + PHI[0]
        rows = (q & 1) * 64
        cols = MARG + (q >> 1)
        xi0[core, rows[:, None] + ch[None, :], cols[:, None]] = feat[sel]
        # occupancy masks at phases 2 (layer-1 out) and 1 (layer-2 out)
        for mk, phi in ((m1, PHI[1]), (m2, PHI[2])):
            qq = s + phi
            mk[core, ((qq & 1) * 64)[:, None] + ch[None, :],
               (MARG + (qq >> 1))[:, None]] = 1.0

    # weight stacks: per layer, 6 stationaries of [contract 128, out 128]
    def k_of(dxs, dys):
        return 3 * (dxs + 1) + (dys + 1)

    mats = []
    for W in Ws:  # [9, 64, 64] (k, c_in, c_out)
        for dxs in (-1, 0, 1):
            for j in (0, 1):
                M = np.zeros((128, 128), np.float32)
                if j == 0:
                    M[0:64, 0:64] = W[k_of(dxs, -1)]      # A: even-in -> even-out
                    M[64:128, 0:64] = W[k_of(dxs, 0)]     # C: odd-in  -> even-out
                    M[64:128, 64:128] = W[k_of(dxs, -1)]  # D: odd-in  -> odd-out
                else:
                    M[0:64, 0:64] = W[k_of(dxs, 1)]       # A
                    M[0:64, 64:128] = W[k_of(dxs, 0)]     # B: even-in -> odd-out
                    M[64:128, 64:128] = W[k_of(dxs, 1)]   # D
                mats.append(M)
    wstack = np.stack(mats).transpose(1, 0, 2).reshape(128, 18 * 128)

    bnv = np.zeros((128, 8), np.float32)
    for l in range(3):
        bnv[0:64, l] = scales[l]
        bnv[64:128, l] = scales[l]
        bnv[0:64, 3 + l] = biases[l]
        bnv[64:128, 3 + l] = biases[l]

    mask_dt = np.float16 if np_dt == np.float16 else _BF16
    return (xi0.astype(np_dt), m1.astype(mask_dt),
            m2.astype(mask_dt), wstack.astype(np_dt), bnv)


def _build_program(dt_key, loop_n=0, variant="v2", psum_bufs=8, warmup=10,
                   hints=True, stagger=False, unroll=8, stage_marks=True):
    import concourse.tile as tile
    from concourse import bacc, mybir

    f32 = mybir.dt.float32
    f16 = mybir.dt.float16
    if dt_key == "bf16":
        DT = mybir.dt.bfloat16
        BF = mybir.dt.bfloat16
    elif dt_key == "fp16":
        DT = mybir.dt.float16
        BF = mybir.dt.float16
    else:
        raise ValueError(dt_key)

    nc = bacc.Bacc("TRN2", target_bir_lowering=False, debug=False,
                   num_devices=NCORES)
    xi0_d = nc.dram_tensor("xi0", [128, WBUF], DT, kind="ExternalInput").ap()
    m1_d = nc.dram_tensor("m1", [128, WBUF], BF, kind="ExternalInput").ap()
    m2_d = nc.dram_tensor("m2", [128, WBUF], BF, kind="ExternalInput").ap()
    wts_d = nc.dram_tensor("wts", [128, 18 * 128], DT, kind="ExternalInput").ap()
    bnv_d = nc.dram_tensor("bnv", [128, 8], f32, kind="ExternalInput").ap()
    out_d = nc.dram_tensor("out", [128, OUTW], f16, kind="ExternalOutput").ap()

    Relu = mybir.ActivationFunctionType.Relu
    mult = mybir.AluOpType.mult

    WIN = [_windows(L1R), _windows(L2R), _windows(L3R)]

    with tile.TileContext(nc) as tc:
        with (
            tc.tile_pool(name="big", bufs=1) as big,
            tc.tile_pool(name="psum", bufs=psum_bufs, space="PSUM") as psump,
            tc.tile_pool(name="tmp", bufs=6) as tmpp,
        ):
            xa = big.tile([128, WBUF], DT)
            xb = big.tile([128, WBUF], DT)
            xc = big.tile([128, WBUF], DT)
            x3 = big.tile([128, OUTW], f16)
            m1t = big.tile([128, WBUF], BF)
            m2t = big.tile([128, WBUF], BF)
            wt = big.tile([128, 18 * 128], DT)
            bnt = big.tile([128, 8], f32)
            scr = big.tile([128, 640], DT)

            def dma_chunks(dst, src, edges, eng):
                for a, bnd in zip(edges[:-1], edges[1:]):
                    eng.dma_start(out=dst[:, a:bnd], in_=src[:, a:bnd])

            # xi0 chunk edges: leading chunks small so L1 window 0 (reads
            # cols [113, 724)) can start early.
            xi_edges = [0, 760, 1480, 2560, 3680, 4800, 5920, 7040, 8160, WBUF]
            mk_edges = [0, 2294, 4588, 6882, WBUF]

            def prologue():
                nc.sync.dma_start(out=bnt, in_=bnv_d)
                nc.sync.dma_start(out=wt, in_=wts_d)
                dma_chunks(xa, xi0_d, xi_edges, nc.sync)
                dma_chunks(m1t, m1_d, mk_edges, nc.sync)
                dma_chunks(m2t, m2_d, mk_edges, nc.sync)
                nc.vector.memset(scr, 0.0)
                for _ in range(warmup):
                    wps = psump.tile([128, WCOLS], f32, tag="ps")
                    nc.tensor.matmul(wps, scr[:, 0:128], scr[:, 128:640],
                                     start=True, stop=True)

            def layer(xin, xout, mt, l, prefetch=None, boundary_after=None):
                sc = bnt[:, l:l + 1]
                bi = bnt[:, 3 + l:4 + l]
                wins = WIN[l]
                for wi, (base, wc) in enumerate(wins):
                    if wi == boundary_after:
                        tc.stage_boundary()
                    ps = psump.tile([128, WCOLS], f32, tag="ps")
                    for i, v in enumerate(VOFF):
                        lhsT = wt[:, (6 * l + i) * 128:(6 * l + i + 1) * 128]
                        nc.tensor.matmul(ps[:, 0:wc], lhsT,
                                         xin[:, base + v:base + v + wc],
                                         start=(i == 0), stop=(i == 5))
                    if mt is not None:
                        tm = tmpp.tile([128, WCOLS], DT, tag="tm")
                        nc.scalar.activation(tm[:, 0:wc], ps[:, 0:wc], Relu,
                                             bias=bi, scale=sc)
                        nc.vector.tensor_tensor(
                            out=xout[:, base:base + wc], in0=tm[:, 0:wc],
                            in1=mt[:, base:base + wc], op=mult)
                    else:
                        o0 = base - L3R[0]
                        nc.scalar.activation(x3[:, o0:o0 + wc], ps[:, 0:wc],
                                             Relu, bias=bi, scale=sc)
                        nc.sync.dma_start(out=out_d[:, o0:o0 + wc],
                                          in_=x3[:, o0:o0 + wc])
                if prefetch is not None:
                    prefetch()

            def body(prefetch, staged=False):
                def pf1():
                    if prefetch:
                        dma_chunks(m1t, m1_d, mk_edges, nc.sync)
                        dma_chunks(xa, xi0_d, xi_edges, nc.sync)
                    if staged:
                        tc.stage_boundary()

                def pf2():
                    if prefetch:
                        dma_chunks(m2t, m2_d, mk_edges, nc.sync)
                    if staged:
                        tc.stage_boundary()

                layer(xa, xb, m1t, 0, pf1,
                      boundary_after=9 if staged else None)
                layer(xb, xc, m2t, 1, pf2)
                layer(xc, None, None, 2)

            prologue()
            if loop_n > 0:
                if loop_n % unroll != 0:
                    unroll = 1
                he = (mybir.EngineType.PE,) if hints else ()
                with tc.For_i(0, loop_n // unroll, 1, hint_engines=he,
                              staggered_reset=stagger):
                    for _ in range(unroll):
                        body(prefetch=True, staged=stagger and stage_marks)
            else:
                body(prefetch=False)
    nc.compile()
    return nc


def _get_np_dt(dt_key):
    if dt_key == "bf16":
        import ml_dtypes
        return ml_dtypes.bfloat16
    if dt_key == "fp16":
        return np.float16
    return np.float32


def kernel(feat, coor, kin_idx,
           W1, g1, b1, m1, v1,
           W2, g2, b2, m2, v2,
           W3, g3, b3, m3, v3):
    from concourse import bass_utils

    dt_key = os.environ.get("KERNEL_DT", "fp16")
    np_dt = _get_np_dt(dt_key)

    feat = np.asarray(feat, np.float32)
    coor = np.asarray(coor)
    Ws = [np.asarray(W, np.float32) for W in (W1, W2, W3)]
    scales, biases = [], []
    for g, bb, mm, vv in ((g1, b1, m1, v1), (g2, b2, m2, v2), (g3, b3, m3, v3)):
        s = np.asarray(g, np.float32) / np.sqrt(np.asarray(vv, np.float32) + EPS)
        scales.append(s)
        biases.append(np.asarray(bb, np.float32) - np.asarray(mm, np.float32) * s)

    xi0, m1g, m2g, wstack, bnv = _host_prepare(feat, coor, Ws, scales, biases,
                                               np_dt)

    if dt_key not in _CACHE:
        _CACHE[dt_key] = _build_program(dt_key)
    nc = _CACHE[dt_key]

    in_maps = [
        {"xi0": np.ascontiguousarray(xi0[c]),
         "m1": np.ascontiguousarray(m1g[c]),
         "m2": np.ascontiguousarray(m2g[c]),
         "wts": wstack, "bnv": bnv}
        for c in range(NCORES)
    ]
    res = None
    for attempt in range(3):
        try:
            res = bass_utils.run_bass_kernel_spmd(
                nc, in_maps, core_ids=list(range(NCORES)))
            break
        except Exception:
            if attempt == 2:
                raise
            import time
            time.sleep(5)
    grids = np.stack([np.asarray(r["out"], np.float32)
                      for r in res.results])  # [8, 128, OUTW]
    return _gather(grids, coor)


def _gather(grids, coor):
    """Gather per-voxel rows from the owning core's grid (phase 0)."""
    grids = grids.reshape(NCORES, 2, 64, OUTW)
    xs = (coor[:, 1].astype(np.int64) + XLIM) // 2
    ys = (coor[:, 2].astype(np.int64) + YLIM) // 2
    b = coor[:, 0].astype(np.int64)
    half = (xs >= OWN0).astype(np.int64)
    core = 2 * b + half
    xstart = np.where(half == 0, -4, OWN0 - 4)
    s = (xs - xstart) * YP + ys + 1
    out = grids[core, s & 1, :, (s >> 1) - OUT0].astype(np.float32)  # [N, 64]

    xy_ok = ((coor[:, 1] > -XLIM) & (coor[:, 1] <= XLIM)
             & (coor[:, 2] > -YLIM) & (coor[:, 2] <= YLIM))
    out *= xy_ok[:, None].astype(np.float32)
    return out


_BF16 = None


def _init_bf16():
    global _BF16
    import ml_dtypes
    _BF16 = ml_dtypes.bfloat16


_init_bf16()
